# revision 1
# baseline (speedup 1.0000x reference)
"""Trainium2 Bass kernel for CSWin-style full attention with LePE.

Module (B=2, C=256, H=W=48, heads=8, head_dim=32):
    qkv = conv1x1(x)            -> q, k, v per head
    attn = softmax(k^T q * d^-0.5, over keys)
    out  = v @ attn + lepe(v)   (lepe = depthwise 3x3 conv + bias)
    out  = conv1x1(out) + b_proj

Sharding: 16 (batch, head) units over 8 cores -> each core owns one batch
index and two heads.  Each core computes its heads' attention + lepe and a
partial projection (w_proj columns of its channels); the host sums the 4
partials per batch and adds b_proj.

Per-core pipeline (matmul operands fp16, fp32 PSUM accumulation).  Every
matmul is zero-padded to a full 128x128 tile config (K and M padded with
zero rows/cols, all operands at partition base 0): mixed PE tiling
geometries (32-row s-matmuls, 33/64-col PV matmuls) interleaved with
in-flight full-array matmuls produced nondeterministic PSUM corruption
on hardware — padding is free anyway since every matmul here is bound by
its N-dim streaming, not K/M.

  - qkv 1x1-conv matmuls -> q,k rearranged (SBUF->SBUF DMA) into
    per-unit [128(=32 data + 96 zero), N] q and k slabs.
  - s = k^T q in [128, 512] PSUM chunks; exp on ScalarE straight out of
    PSUM over [128, 1536/768] tiles (softmax max-subtraction skipped:
    s*scale ~ N(0,1), exp cannot overflow); scale rides the activation's
    free affine.  p stored fp16 in SBUF.
  - P*V accumulated over a whole 768-wide window (one v^T weight load
    per k-tile) with a ones-column appended to v^T, so the softmax
    denominator falls out of the same matmul.  Normalization is deferred
    to the [32, q] output: the denominator row is shifted to partition 0
    (DVE copies may change partition base), reciprocated there
    (custom-DVE ops break with PSUM inputs or nonzero partition base),
    broadcast across partitions with a stream shuffle, and multiplied in.
  - lepe as 9 accumulating (shifted-)diag matmuls over a zero-padded
    [*, 50, 50] copy of v (SAME padding free from the zero border);
    b_lepe folded into the PSUM evacuation.
  - proj from a single merged y slab (A rows 0:32, B rows 64:96);
    output evacuation on ScalarE (DVE is the epilogue-bound engine).
  - v/v^T/lepe prep and the previous window's PV+epilogue run as fill
    tasks interleaved into each window's kt loop to keep the PE dense
    (HAM stays warm) while ScalarE streams the exps.
"""

import os

import numpy as np

import concourse.bacc as bacc
import concourse.mybir as mybir
import concourse.tile as tile
from concourse.bass_utils import run_bass_kernel_spmd

F16 = mybir.dt.float16
F32 = mybir.dt.float32
ADD = mybir.AluOpType.add
EXP = mybir.ActivationFunctionType.Exp

B, C, H, W = 2, 256, 48, 48
N = H * W                      # 2304
HEADS, D = 8, 32
SCALE = D ** -0.5
NCORES = 8
KT = N // 128                  # 18 key tiles
QWIN = [(0, 768), (768, 768), (1536, 768)]
SUBW = 384
TAPS = [(dy, dx) for dy in (-1, 0, 1) for dx in (-1, 0, 1)]


def _chunks(total, step):
    out, o = [], 0
    while o < total:
        out.append((o, min(step, total - o)))
        o += step
    return out


def _emit(nc, tc, pools, tensors, dbg=None):
    const, sb, pp, tmpp, rcpp, obp, ps_o, ps_s, ps_b = pools
    x_d, wqk_d, wv_d, dg_d, bl_d, wp_d, out_d = tensors

    # ---- persistent SBUF tensors -----------------------------------
    x_sb = sb.tile([128, 2, N], F16, tag="x")
    # qk4[:, u, 0/1, :]: unit u's q (0) / k (1); rows 32:128 zero
    qk4 = sb.tile([128, 2, 2, N], F16, tag="qk4")
    qk_tmp = sb.tile([128, N], F16, tag="qktmp")
    vpad = sb.tile([128, 50, 50], F16, tag="vpad")      # rows 64:128 zero
    # vT2[:, u, kt, :]: [vT_u (32) | ones (1) | zeros (95)]
    vT2 = sb.tile([128, 2, KT, 128], F16, tag="vT")
    lepe_sb = sb.tile([32, 2, N], F16, tag="lepe")
    y3 = sb.tile([128, N], F16, tag="y")   # A rows 0:32, B rows 64:96

    wqk = const.tile([128, 2, 128], F16, tag="wqk")
    wv = const.tile([128, 2, 128], F16, tag="wv")       # cols 64:128 zero
    dg = const.tile([128, 2, 9, 128], F16, tag="dg")
    bl = const.tile([32, 2], F32, tag="bl")
    wp = const.tile([128, 2, 128], F16, tag="wp")
    # rc32 ping-pong: row 0 holds the per-sub reciprocal; rows 1:32 stay
    # zero so the broadcast shuffle's unused source lanes are finite.
    rc32 = [sb.tile([32, 512], F32, tag=f"rc32{i}", name=f"rc32{i}")
            for i in range(2)]

    for cc in range(2):
        nc.sync.dma_start(wqk[:, cc, :], wqk_d[cc])
    for cc in range(2):
        for h0, hw in _chunks(N, 1152):
            nc.sync.dma_start(x_sb[:, cc, h0:h0 + hw], x_d[cc, :, h0:h0 + hw])
    nc.sync.dma_start(wv[:, :, :], wv_d[:, :, :])
    nc.sync.dma_start(dg[:, :, :, :], dg_d[:, :, :, :])
    nc.sync.dma_start(bl[:, :], bl_d[:, :])
    nc.sync.dma_start(wp[:, :, :], wp_d[:, :, :])

    nc.vector.memset(y3[:], 0.0)
    nc.vector.memset(vpad[:], 0.0)
    nc.vector.memset(qk4[:], 0.0)
    nc.vector.memset(vT2[:], 0.0)
    for u in range(2):
        nc.vector.memset(vT2[:, u, :, 32:33], 1.0)
    for rv in rc32:
        nc.vector.memset(rv[:], 0.0)

    # ---- qkv: q/k block [128, N] -> qk_tmp -> qk4 ------------------
    for c0, cw in _chunks(N, 512):
        t = ps_o.tile([128, 512], F32, tag="po")
        nc.tensor.matmul(t[:, :cw], wqk[:, 0, :], x_sb[:, 0, c0:c0 + cw],
                         start=True, stop=False)
        nc.tensor.matmul(t[:, :cw], wqk[:, 1, :], x_sb[:, 1, c0:c0 + cw],
                         start=False, stop=True)
        nc.vector.tensor_copy(qk_tmp[:, c0:c0 + cw], t[:, :cw])
    # qk_tmp rows: qA 0:32 | kA 32:64 | qB 64:96 | kB 96:128
    for u in range(2):
        for h0, hw in _chunks(N, 1152):
            nc.sync.dma_start(qk4[0:32, u, 0, h0:h0 + hw],
                              qk_tmp[64 * u:64 * u + 32, h0:h0 + hw])
            nc.sync.dma_start(qk4[0:32, u, 1, h0:h0 + hw],
                              qk_tmp[64 * u + 32:64 * u + 64, h0:h0 + hw])

    # ---- early-phase fill tasks (run interleaved into window A1) ---
    def v_task(r0, nr):
        def task():
            c0, cw = r0 * W, nr * W
            t = ps_o.tile([128, 512], F32, tag="po")
            nc.tensor.matmul(t[:, :cw], wv[:, 0, :], x_sb[:, 0, c0:c0 + cw],
                             start=True, stop=False)
            nc.tensor.matmul(t[:, :cw], wv[:, 1, :], x_sb[:, 1, c0:c0 + cw],
                             start=False, stop=True)
            nc.vector.tensor_copy(vpad[0:64, 1 + r0:1 + r0 + nr, 1:49],
                                  t[0:64, :cw])
        return task

    def vt_task(nts):
        def task():
            for nt in nts:
                t = ps_o.tile([128, 512], F32, tag="po")
                nc.tensor.matmul(t[:, 0:128],
                                 x_sb[:, 0, nt * 128:(nt + 1) * 128],
                                 wv[:, 0, :], start=True, stop=False)
                nc.tensor.matmul(t[:, 0:128],
                                 x_sb[:, 1, nt * 128:(nt + 1) * 128],
                                 wv[:, 1, :], start=False, stop=True)
                nc.vector.tensor_copy(vT2[:, 0, nt, 0:32], t[:, 0:32])
                nc.vector.tensor_copy(vT2[:, 1, nt, 0:32], t[:, 32:64])
        return task

    def lepe_task(chunks, u):
        def task():
            ts = [ps_o.tile([128, 512], F32, tag="po", name=f"lp{u}_{i}")
                  for i, _ in enumerate(chunks)]
            for ti, (dy, dx) in enumerate(TAPS):
                for t, (r0, nr) in zip(ts, chunks):
                    nc.tensor.matmul(
                        t[:, :nr * W], dg[:, u, ti, :],
                        vpad[:, 1 + r0 + dy:1 + r0 + dy + nr, 1 + dx:49 + dx],
                        start=(ti == 0), stop=(ti == 8))
            for t, (r0, nr) in zip(ts, chunks):
                nc.vector.tensor_scalar(
                    lepe_sb[0:32, u, r0 * W:(r0 + nr) * W], t[0:32, :nr * W],
                    bl[0:32, u:u + 1], None, ADD)
        return task

    early = ([v_task(r0, nr) for r0, nr in _chunks(H, 10)]
             + [vt_task(list(range(n, min(n + 6, KT)))) for n in range(0, KT, 6)]
             + [lepe_task(pair, u) for u in range(2)
                for pair in ([(0, 10), (10, 10)], [(20, 10), (30, 10)],
                             [(40, 8)])])

    # ---- attention -------------------------------------------------
    def pv_window(u, p_t, q0, si0):
        """PV over a whole 768 window (one v^T load per k-tile), then
        normalize/combine/proj per 384 sub-chunk."""
        ob_ = ps_b.tile([128, 768], F32, tag="pb")
        for kt in range(KT):
            for s0, sw in _chunks(768, 512):
                nc.tensor.matmul(ob_[:, s0:s0 + sw], vT2[:, u, kt, :],
                                 p_t[:, kt, s0:s0 + sw],
                                 start=(kt == 0), stop=(kt == KT - 1))
        for i, (s0, sw) in enumerate(_chunks(768, SUBW)):
            pv_sub(u, ob_, q0, s0, sw, si0 + i)

    def pv_sub(u, o, q0, s0, sw, si):
        o = o[:, s0:s0 + 512] if sw == 512 else o[:, s0:s0 + sw]
        # custom-DVE ops misbehave at nonzero partition base and with PSUM
        # inputs: shift the denominator row down to partition 0 in SBUF
        # (DVE copies may change partition base), recip there, broadcast
        # across 32 partitions with a stream shuffle.
        rv = rc32[si % 2]
        dn = rcpp.tile([1, 512], F32, tag="dn")
        nc.vector.tensor_copy(dn[0:1, :sw], o[32:33, :sw])
        osb = tmpp.tile([32, 512], F32, tag="osb")
        nc.vector.tensor_copy(osb[0:32, :sw], o[0:32, :sw])
        nc.vector.reciprocal_approx_fast(rv[0:1, :sw], dn[0:1, :sw])
        rbs = rcpp.tile([32, 512], F32, tag="rcp")
        nc.vector.stream_shuffle(rbs[0:32, :sw], rv[0:32, :sw], [0] * 32)
        tm = tmpp.tile([32, 512], F32, tag="tmp")
        nc.vector.tensor_mul(tm[0:32, :sw], osb[0:32, :sw], rbs[0:32, :sw])
        qs = q0 + s0
        if u == 0:
            nc.vector.tensor_add(y3[0:32, qs:qs + sw],
                                 lepe_sb[0:32, 0, qs:qs + sw], tm[0:32, :sw])
        else:
            tmB = tmpp.tile([32, 512], F16, tag="tmB")
            nc.vector.tensor_add(tmB[0:32, :sw], lepe_sb[0:32, 1, qs:qs + sw],
                                 tm[0:32, :sw])
            nc.vector.tensor_copy(y3[64:96, qs:qs + sw], tmB[0:32, :sw])
            proj_sub(qs, sw)
        if dbg is not None:
            nc.sync.dma_start(dbg["rcp"][u, si, 0:1, :sw], rv[0:1, :sw])
            nc.sync.dma_start(dbg["rb"][u, si, :, :sw], rbs[0:32, :sw])
            nc.sync.dma_start(dbg["tm"][u, si, :, :sw], tm[0:32, :sw])

    def proj_sub(qs, sw):
        for mc in range(2):
            po = ps_o.tile([128, 512], F32, tag="po")
            nc.tensor.matmul(po[:, :sw], wp[:, mc, :], y3[:, qs:qs + sw],
                             start=True, stop=True)
            ob = obp.tile([128, 512], F32, tag="ob")
            nc.scalar.copy(ob[:, :sw], po[:, :sw])
            nc.sync.dma_start(out_d[mc, :, qs:qs + sw], ob[:, :sw])

    def make_task(u, p_t, q0, si0):
        def task():
            pv_window(u, p_t, q0, si0)
        return task

    pending = early
    sic = {0: 0, 1: 0}
    for u in range(2):
        for q0, qw in QWIN:
            p_t = pp.tile([128, KT, qw], F16, tag="p")
            work, pending = list(pending), []
            every = max(1, KT // max(1, len(work)))
            for kt in range(KT):
                s_t = ps_s.tile([128, qw], F32, tag="s")
                for s0, sw in _chunks(qw, 512):
                    nc.tensor.matmul(s_t[:, s0:s0 + sw],
                                     qk4[:, u, 1, kt * 128:(kt + 1) * 128],
                                     qk4[:, u, 0, q0 + s0:q0 + s0 + sw],
                                     start=True, stop=True)
                nc.scalar.activation(p_t[:, kt, :], s_t[:, :], EXP, scale=SCALE)
                if work and kt % every == every - 1:
                    work.pop(0)()
            for fn in work:
                fn()
            if dbg is not None:
                nc.sync.dma_start(dbg["p"][u, :, :, q0:q0 + qw], p_t[:, :, :])
            pending = [make_task(u, p_t, q0, sic[u])]
            sic[u] += len(_chunks(qw, SUBW))
    for fn in pending:
        fn()
    if dbg is not None:
        nc.sync.dma_start(dbg["y"][:, :], y3[:, :])
        nc.sync.dma_start(dbg["qkt"][:, :], qk_tmp[:, :])
        nc.sync.dma_start(dbg["qk4"][:, :, :, :], qk4[:, :, :, :])
        nc.sync.dma_start(dbg["vpad"][:, :, :], vpad[:, :, :])
        nc.sync.dma_start(dbg["vT"][:, :, :, :], vT2[:, :, :, :])
        nc.sync.dma_start(dbg["lepe"][:, :, :], lepe_sb[:, :, :])


def _build():
    nc = bacc.Bacc("TRN2", target_bir_lowering=False, debug=False)

    x_d = nc.dram_tensor("x", [2, 128, N], F16, kind="ExternalInput")
    wqk_d = nc.dram_tensor("wqk", [2, 128, 128], F16, kind="ExternalInput")
    wv_d = nc.dram_tensor("wv", [128, 2, 128], F16, kind="ExternalInput")
    dg_d = nc.dram_tensor("dg", [128, 2, 9, 128], F16, kind="ExternalInput")
    bl_d = nc.dram_tensor("bl", [32, 2], F32, kind="ExternalInput")
    wp_d = nc.dram_tensor("wp", [128, 2, 128], F16, kind="ExternalInput")
    out_d = nc.dram_tensor("out", [2, 128, N], F32, kind="ExternalOutput")
    dbg = None
    if os.environ.get("KDBG"):
        dbg = {
            "rcp": nc.dram_tensor("dbg_rcp", [2, 6, 1, 512], F32,
                                  kind="ExternalOutput"),
            "rb": nc.dram_tensor("dbg_rb", [2, 6, 32, 512], F32,
                                 kind="ExternalOutput"),
            "tm": nc.dram_tensor("dbg_tm", [2, 6, 32, 512], F32,
                                 kind="ExternalOutput"),
            "p": nc.dram_tensor("dbg_p", [2, 128, KT, N], F16,
                                kind="ExternalOutput"),
            "y": nc.dram_tensor("dbg_y", [128, N], F16,
                                kind="ExternalOutput"),
            "qkt": nc.dram_tensor("dbg_qkt", [128, N], F16,
                                  kind="ExternalOutput"),
            "qk4": nc.dram_tensor("dbg_qk4", [128, 2, 2, N], F16,
                                  kind="ExternalOutput"),
            "vpad": nc.dram_tensor("dbg_vpad", [128, 50, 50], F16,
                                   kind="ExternalOutput"),
            "vT": nc.dram_tensor("dbg_vT", [128, 2, KT, 128], F16,
                                 kind="ExternalOutput"),
            "lepe": nc.dram_tensor("dbg_lepe", [32, 2, N], F16,
                                   kind="ExternalOutput"),
        }

    with tile.TileContext(nc) as tc:
        with (
            tc.tile_pool(name="const", bufs=1) as const,
            tc.tile_pool(name="sb", bufs=1) as sb,
            tc.tile_pool(name="pp", bufs=2) as pp,
            tc.tile_pool(name="tmp", bufs=2) as tmpp,
            tc.tile_pool(name="rcp", bufs=2) as rcpp,
            tc.tile_pool(name="ob", bufs=4) as obp,
            tc.tile_pool(name="ps_o", bufs=2, space="PSUM") as ps_o,
            tc.tile_pool(name="ps_s", bufs=2, space="PSUM") as ps_s,
            tc.tile_pool(name="ps_b", bufs=1, space="PSUM") as ps_b,
        ):
            _emit(nc, tc,
                  (const, sb, pp, tmpp, rcpp, obp, ps_o, ps_s, ps_b),
                  (x_d, wqk_d, wv_d, dg_d, bl_d, wp_d, out_d), dbg=dbg)

    nc.compile()
    return nc


_NC = None


def _get_nc():
    global _NC
    if _NC is None:
        _NC = _build()
    return _NC


def _prep_core(c, x, w_qkv, w_lepe, b_lepe, w_proj):
    b = c // 4
    hA, hB = 2 * (c % 4), 2 * (c % 4) + 1
    xb = np.asarray(x[b], np.float32).reshape(C, N)
    w_qkv = np.asarray(w_qkv, np.float32)
    w_lepe = np.asarray(w_lepe, np.float32)
    b_lepe = np.asarray(b_lepe, np.float32)
    w_proj = np.asarray(w_proj, np.float32)

    rows = np.concatenate([
        w_qkv[96 * hA + 0:96 * hA + 32],       # qA
        w_qkv[96 * hA + 32:96 * hA + 64],      # kA
        w_qkv[96 * hB + 0:96 * hB + 32],       # qB
        w_qkv[96 * hB + 32:96 * hB + 64],      # kB
    ], axis=0)                                 # [128, 256]
    wqk = np.ascontiguousarray(rows.T.reshape(2, 128, 128)).astype(np.float16)

    # wv[c', cc, j]: v weights for both units, transposed; cols 64:128 zero
    wv = np.zeros((2, 128, 128), np.float32)
    wv[:, :, 0:32] = w_qkv[96 * hA + 64:96 * hA + 96].T.reshape(2, 128, 32)
    wv[:, :, 32:64] = w_qkv[96 * hB + 64:96 * hB + 96].T.reshape(2, 128, 32)
    wv = np.ascontiguousarray(wv.transpose(1, 0, 2)).astype(np.float16)

    # dg[c', u, ti, c]: shifted diag; unit u channel c lives at vpad row
    # 32*u + c
    dg = np.zeros((128, 2, 9, 128), np.float32)
    idx = np.arange(32)
    for ti, (dy, dx) in enumerate(TAPS):
        dg[idx, 0, ti, idx] = w_lepe[32 * hA:32 * hA + 32, 0, dy + 1, dx + 1]
        dg[32 + idx, 1, ti, idx] = w_lepe[32 * hB:32 * hB + 32, 0,
                                          dy + 1, dx + 1]
    dg = dg.astype(np.float16)

    bl = np.zeros((32, 2), np.float32)
    bl[:, 0] = b_lepe[32 * hA:32 * hA + 32]
    bl[:, 1] = b_lepe[32 * hB:32 * hB + 32]

    # wp[c', mc, o']: proj weights; y rows A 0:32, B 64:96, rest zero
    wp = np.zeros((128, 2, 128), np.float32)
    wp[0:32] = w_proj[:, 32 * hA:32 * hA + 32].T.reshape(32, 2, 128)
    wp[64:96] = w_proj[:, 32 * hB:32 * hB + 32].T.reshape(32, 2, 128)
    wp = wp.astype(np.float16)

    return {
        "x": np.ascontiguousarray(xb.reshape(2, 128, N)).astype(np.float16),
        "wqk": wqk, "wv": wv, "dg": dg, "bl": bl, "wp": wp,
    }


_LAST_RES = None


def kernel(x, w_qkv, w_lepe, b_lepe, w_proj, b_proj, **_ignored):
    global _LAST_RES
    nc = _get_nc()
    in_maps = [_prep_core(c, x, w_qkv, w_lepe, b_lepe, w_proj)
               for c in range(NCORES)]
    res = run_bass_kernel_spmd(nc, in_maps, core_ids=list(range(NCORES)))
    _LAST_RES = res
    out = np.zeros((B, C, N), np.float32)
    for c in range(NCORES):
        out[c // 4] += res.results[c]["out"].reshape(C, N)
    out += np.asarray(b_proj, np.float32)[None, :, None]
    return out.reshape(B, C, H, W).astype(np.float32)



# revision 3
# speedup vs baseline: 1.0212x; 1.0212x over previous
"""Trainium2 Bass kernel for CSWin-style full attention with LePE.

Module (B=2, C=256, H=W=48, heads=8, head_dim=32):
    qkv = conv1x1(x)            -> q, k, v per head
    attn = softmax(k^T q * d^-0.5, over keys)
    out  = v @ attn + lepe(v)   (lepe = depthwise 3x3 conv + bias)
    out  = conv1x1(out) + b_proj

Sharding: 16 (batch, head) units over 8 cores -> each core owns one batch
index and two heads (A, B).  Each core computes its heads' attention +
lepe and a partial projection (w_proj columns of its channels); the host
sums the 4 partials per batch and adds b_proj.

v2 design (vs the padded-128x128 baseline at ~158us):
  - S = k^T q matmuls use PE row tiling (tile_position=(32i,0), 32x128
    mode): groups of 4/2 k-tiles run concurrently on disjoint 32-row
    bands, each streaming its own replica of q.  q and k are replicated
    at 4 partition bases by SBUF->SBUF DMA out of the qkv output.
  - exp(s*scale) is split between ScalarE (exact activation, multi-bank
    [128, nk*512] PSUM reads) and the DVE (Schraudolph: bits =
    rne(s*K0 + K1) as int16 IS fp16 2^y, ~3% max rel err; one stock
    tensor_scalar per group, output bitcast into the fp16 p slab).
    Assignment string ENG balances the two engine queues.
  - PV uses PE column tiling (128x64 mode): unit A accumulates into PSUM
    partitions 0:33, unit B into 64:97 of the same bank, concurrently.
    vT carries an appended ones column so the softmax denominator falls
    out of the same matmuls (row 32 / 96).
  - lepe is a single merged block-diagonal 9-tap matmul stream over a
    zero-padded [*, 50, 50] copy of v covering both units (A -> out
    partitions 0:32 from vpad rows 0:32, B -> 64:96 from rows 32:64).
  - Normalization: dens copied to partitions 0/64, reciprocal_approx,
    one group-local stream_shuffle broadcast (mask [0]*32: partitions
    0:32 <- 0, 64:96 <- 64), multiply straight out of PV PSUM, add lepe.
    Unused PV PSUM rows are zeroed once so the junk rows stay finite
    (proj weights are zero there).
  - PSUM banks: S ping [128,4,512] banks 0-3, S pong [128,2,512] 4-5,
    PV + proj share banks 6/7.  Phase 0 (qkv/v/vT/lepe) rotates its
    scratch through the S banks before the chunk loop starts.
"""

import numpy as np

import concourse.bacc as bacc
import concourse.mybir as mybir
import concourse.tile as tile
from concourse.bass_utils import run_bass_kernel_spmd

F16 = mybir.dt.float16
F32 = mybir.dt.float32
I16 = mybir.dt.int16
ADD = mybir.AluOpType.add
MULT = mybir.AluOpType.mult
EXP = mybir.ActivationFunctionType.Exp

B, C, H, W = 2, 256, 48, 48
N = H * W                      # 2304
HEADS, D = 8, 32
SCALE = D ** -0.5
NCORES = 8
KT = N // 128                  # 18 key tiles
TAPS = [(dy, dx) for dy in (-1, 0, 1) for dx in (-1, 0, 1)]

# Schraudolph fp16 exp2 bits: rne(y*1024 + (15 - 0.043033)*1024), y in
# log2 units; K0 folds the s * d^-0.5 * log2(e) scaling.
K0 = SCALE * 1.4426950408889634 * 1024.0
K1 = (15.0 - 0.043033) * 1024.0

CHUNKS = [(0, 512), (512, 512), (1024, 512), (1536, 512), (2048, 256)]
# (kt0, nk, 0=ping banks 0-3 / 1=pong banks 4-5)
SGROUPS = [(0, 4, 0), (4, 2, 1), (6, 4, 0), (10, 2, 1), (12, 4, 0),
           (16, 2, 1)]
# exp engine per S-group: A=ScalarE activation, V=DVE Schraudolph
ENG = "AVAVVA"


def _chunks(total, step):
    out, o = [], 0
    while o < total:
        out.append((o, min(step, total - o)))
        o += step
    return out


def _emit(nc, tc, pools, tensors):
    const, sb, obp, ps_a, ps_b, ps_v = pools
    x_d, wqk_d, wv_d, dg_d, bl_d, wp_d, out_d = tensors

    # ---- persistent SBUF tensors -----------------------------------
    x_sb = sb.tile([128, 2, N], F16, tag="x")
    qk_tmp = sb.tile([128, N], F16, tag="qktmp")
    qrep = sb.tile([128, 2, N], F16, tag="qrep")
    krep = sb.tile([128, 2, N], F16, tag="krep")
    vpad = sb.tile([128, 50, 50], F16, tag="vpad")      # rows 64:128 zero
    # vT2[:, kt, u, 0:32] = vT, [:, kt, u, 32] = ones (denominator)
    vT2 = sb.tile([128, KT, 2, 33], F16, tag="vT")
    lepe128 = sb.tile([128, N], F16, tag="lepe")        # A 0:32, B 64:96
    # p slabs, double buffered by chunk parity
    p_t = [[sb.tile([128, KT, 512], F16, tag=f"p{u}{pb}",
                    name=f"p{u}{pb}") for pb in range(2)] for u in range(2)]
    y3 = sb.tile([128, 512], F16, tag="y")
    dn128 = sb.tile([128, 512], F32, tag="dn")
    rc128 = sb.tile([128, 512], F32, tag="rc")
    rbs = sb.tile([128, 512], F32, tag="rbs")
    tm = sb.tile([128, 512], F16, tag="tm")

    wqk = const.tile([128, 2, 128], F16, tag="wqk")
    wv = const.tile([128, 2, 128], F16, tag="wv")       # cols 64:128 zero
    dg = const.tile([128, 9, 128], F16, tag="dg")       # merged block-diag
    bl = const.tile([128, 1], F32, tag="bl")
    wp = const.tile([128, 2, 128], F16, tag="wp")

    for cc in range(2):
        nc.sync.dma_start(wqk[:, cc, :], wqk_d[cc])
    for cc in range(2):
        for h0, hw in _chunks(N, 1152):
            nc.sync.dma_start(x_sb[:, cc, h0:h0 + hw], x_d[cc, :, h0:h0 + hw])
    nc.sync.dma_start(wv[:, :, :], wv_d[:, :, :])
    nc.sync.dma_start(dg[:, :, :], dg_d[:, :, :])
    nc.sync.dma_start(bl[:, :], bl_d[:, :])
    nc.sync.dma_start(wp[:, :, :], wp_d[:, :, :])

    nc.vector.memset(vpad[:], 0.0)
    nc.vector.memset(vT2[:], 0.0)
    nc.vector.memset(vT2[:, :, :, 32:33], 1.0)
    nc.vector.memset(dn128[:], 1.0)

    # ---- PSUM layout -----------------------------------------------
    sA = ps_a.tile([128, 4, 512], F32, tag="sA")        # banks 0-3
    sB = ps_b.tile([128, 2, 512], F32, tag="sB")        # banks 4-5
    pvo = ps_v.tile([128, 2, 512], F32, tag="pv")       # banks 6 (pv), 7 (proj)
    pv = pvo[:, 0, :]
    pso = pvo[:, 1, :]
    # junk rows of the PV bank must be finite zeros (read by the
    # epilogue multiply; proj weights are zero there)
    nc.vector.memset(pv[32:64, :], 0.0)
    nc.vector.memset(pv[96:128, :], 0.0)

    ph0 = [sA[:, i, :] for i in range(4)] + [sB[:, i, :] for i in range(2)]
    slot = [0]

    def scratch():
        s = ph0[slot[0] % 6]
        slot[0] += 1
        return s

    # ---- phase 0: qkv -> qk_tmp, replicate, v, vT, lepe ------------
    for c0, cw in _chunks(N, 512):
        t = scratch()
        nc.tensor.matmul(t[:, :cw], wqk[:, 0, :], x_sb[:, 0, c0:c0 + cw],
                         start=True, stop=False)
        nc.tensor.matmul(t[:, :cw], wqk[:, 1, :], x_sb[:, 1, c0:c0 + cw],
                         start=False, stop=True)
        nc.scalar.copy(qk_tmp[:, c0:c0 + cw], t[:, :cw])
    # qk_tmp rows: qA 0:32 | kA 32:64 | qB 64:96 | kB 96:128
    for u in range(2):
        for i in range(4):
            for h0, hw in _chunks(N, 1152):
                nc.sync.dma_start(qrep[32 * i:32 * i + 32, u, h0:h0 + hw],
                                  qk_tmp[64 * u:64 * u + 32, h0:h0 + hw])
                nc.sync.dma_start(krep[32 * i:32 * i + 32, u, h0:h0 + hw],
                                  qk_tmp[64 * u + 32:64 * u + 64, h0:h0 + hw])

    for r0, nr in _chunks(H, 10):
        c0, cw = r0 * W, nr * W
        t = scratch()
        nc.tensor.matmul(t[:, :cw], wv[:, 0, :], x_sb[:, 0, c0:c0 + cw],
                         start=True, stop=False)
        nc.tensor.matmul(t[:, :cw], wv[:, 1, :], x_sb[:, 1, c0:c0 + cw],
                         start=False, stop=True)
        nc.vector.tensor_copy(vpad[0:64, 1 + r0:1 + r0 + nr, 1:49],
                              t[0:64, :cw])

    for nt in range(KT):
        t = scratch()
        nc.tensor.matmul(t[:, 0:128], x_sb[:, 0, nt * 128:(nt + 1) * 128],
                         wv[:, 0, :], start=True, stop=False)
        nc.tensor.matmul(t[:, 0:128], x_sb[:, 1, nt * 128:(nt + 1) * 128],
                         wv[:, 1, :], start=False, stop=True)
        nc.vector.tensor_copy(vT2[:, nt, 0, 0:32], t[:, 0:32])
        nc.vector.tensor_copy(vT2[:, nt, 1, 0:32], t[:, 32:64])

    for r0, nr in _chunks(H, 10):
        t = scratch()
        for ti, (dy, dx) in enumerate(TAPS):
            nc.tensor.matmul(
                t[:, :nr * W], dg[:, ti, :],
                vpad[:, 1 + r0 + dy:1 + r0 + dy + nr, 1 + dx:49 + dx],
                start=(ti == 0), stop=(ti == 8))
        nc.vector.tensor_scalar(lepe128[:, r0 * W:(r0 + nr) * W],
                                t[:, :nr * W], bl[:, 0:1], None, ADD)

    # ---- attention chunks ------------------------------------------
    def emit_s_exp(ci):
        q0, cw = CHUNKS[ci]
        for u in range(2):
            pt = p_t[u][ci % 2]
            for gi, (kt0, nk, bs) in enumerate(SGROUPS):
                ps = sA if bs == 0 else sB
                for j in range(nk):
                    kt = kt0 + j
                    nc.tensor.matmul(
                        ps[:, j, :cw],
                        krep[32 * j:32 * j + 32, u, kt * 128:(kt + 1) * 128],
                        qrep[32 * j:32 * j + 32, u, q0:q0 + cw],
                        start=True, stop=True, tile_position=(32 * j, 0))
                if ENG[gi] == "A":
                    nc.scalar.activation(pt[:, kt0:kt0 + nk, :cw],
                                         ps[:, 0:nk, :cw], EXP, scale=SCALE)
                else:
                    nc.vector.tensor_scalar(
                        pt[:, kt0:kt0 + nk, :cw].bitcast(I16),
                        ps[:, 0:nk, :cw], K0, K1, MULT, ADD)

    def emit_pv_epi_proj(ci):
        q0, cw = CHUNKS[ci]
        for kt in range(KT):
            for u in range(2):
                nc.tensor.matmul(pv[64 * u:64 * u + 33, :cw],
                                 vT2[:, kt, u, 0:33],
                                 p_t[u][ci % 2][:, kt, :cw],
                                 start=(kt == 0), stop=(kt == KT - 1),
                                 tile_position=(0, 64 * u))
        nc.vector.tensor_copy(dn128[0:1, :cw], pv[32:33, :cw])
        nc.vector.tensor_copy(dn128[64:65, :cw], pv[96:97, :cw])
        nc.vector.reciprocal_approx_fast(rc128[:, :cw], dn128[:, :cw])
        nc.vector.stream_shuffle(rbs[:, :cw], rc128[:, :cw], [0] * 32)
        nc.vector.tensor_mul(tm[:, :cw], pv[:, :cw], rbs[:, :cw])
        nc.vector.tensor_add(y3[:, :cw], tm[:, :cw], lepe128[:, q0:q0 + cw])
        for mc in range(2):
            nc.tensor.matmul(pso[:, :cw], wp[:, mc, :], y3[:, :cw],
                             start=True, stop=True)
            ob = obp.tile([128, 512], F32, tag="ob")
            nc.scalar.copy(ob[:, :cw], pso[:, :cw])
            nc.sync.dma_start(out_d[mc, :, q0:q0 + cw], ob[:, :cw])

    for ci in range(len(CHUNKS)):
        emit_s_exp(ci)
        if ci > 0:
            emit_pv_epi_proj(ci - 1)
    emit_pv_epi_proj(len(CHUNKS) - 1)


def _build():
    nc = bacc.Bacc("TRN2", target_bir_lowering=False, debug=False)

    x_d = nc.dram_tensor("x", [2, 128, N], F16, kind="ExternalInput")
    wqk_d = nc.dram_tensor("wqk", [2, 128, 128], F16, kind="ExternalInput")
    wv_d = nc.dram_tensor("wv", [128, 2, 128], F16, kind="ExternalInput")
    dg_d = nc.dram_tensor("dg", [128, 9, 128], F16, kind="ExternalInput")
    bl_d = nc.dram_tensor("bl", [128, 1], F32, kind="ExternalInput")
    wp_d = nc.dram_tensor("wp", [128, 2, 128], F16, kind="ExternalInput")
    out_d = nc.dram_tensor("out", [2, 128, N], F32, kind="ExternalOutput")

    with tile.TileContext(nc) as tc:
        with (
            tc.tile_pool(name="const", bufs=1) as const,
            tc.tile_pool(name="sb", bufs=1) as sb,
            tc.tile_pool(name="ob", bufs=4) as obp,
            tc.tile_pool(name="ps_a", bufs=1, space="PSUM") as ps_a,
            tc.tile_pool(name="ps_b", bufs=1, space="PSUM") as ps_b,
            tc.tile_pool(name="ps_v", bufs=1, space="PSUM") as ps_v,
        ):
            _emit(nc, tc, (const, sb, obp, ps_a, ps_b, ps_v),
                  (x_d, wqk_d, wv_d, dg_d, bl_d, wp_d, out_d))

    nc.compile()
    return nc


_NC = None


def _get_nc():
    global _NC
    if _NC is None:
        _NC = _build()
    return _NC


def _prep_core(c, x, w_qkv, w_lepe, b_lepe, w_proj):
    b = c // 4
    hA, hB = 2 * (c % 4), 2 * (c % 4) + 1
    xb = np.asarray(x[b], np.float32).reshape(C, N)
    w_qkv = np.asarray(w_qkv, np.float32)
    w_lepe = np.asarray(w_lepe, np.float32)
    b_lepe = np.asarray(b_lepe, np.float32)
    w_proj = np.asarray(w_proj, np.float32)

    rows = np.concatenate([
        w_qkv[96 * hA + 0:96 * hA + 32],       # qA
        w_qkv[96 * hA + 32:96 * hA + 64],      # kA
        w_qkv[96 * hB + 0:96 * hB + 32],       # qB
        w_qkv[96 * hB + 32:96 * hB + 64],      # kB
    ], axis=0)                                 # [128, 256]
    wqk = np.ascontiguousarray(rows.T.reshape(2, 128, 128)).astype(np.float16)

    # wv[c', cc, j]: v weights for both units, transposed; cols 64:128 zero
    wv = np.zeros((2, 128, 128), np.float32)
    wv[:, :, 0:32] = w_qkv[96 * hA + 64:96 * hA + 96].T.reshape(2, 128, 32)
    wv[:, :, 32:64] = w_qkv[96 * hB + 64:96 * hB + 96].T.reshape(2, 128, 32)
    wv = np.ascontiguousarray(wv.transpose(1, 0, 2)).astype(np.float16)

    # merged block-diag lepe: A vpad rows 0:32 -> out 0:32, B rows
    # 32:64 -> out 64:96
    dg = np.zeros((128, 9, 128), np.float32)
    idx = np.arange(32)
    for ti, (dy, dx) in enumerate(TAPS):
        dg[idx, ti, idx] = w_lepe[32 * hA:32 * hA + 32, 0, dy + 1, dx + 1]
        dg[32 + idx, ti, 64 + idx] = w_lepe[32 * hB:32 * hB + 32, 0,
                                            dy + 1, dx + 1]
    dg = dg.astype(np.float16)

    bl = np.zeros((128, 1), np.float32)
    bl[0:32, 0] = b_lepe[32 * hA:32 * hA + 32]
    bl[64:96, 0] = b_lepe[32 * hB:32 * hB + 32]

    # wp[c', mc, o']: proj weights; y rows A 0:32, B 64:96, rest zero
    wp = np.zeros((128, 2, 128), np.float32)
    wp[0:32] = w_proj[:, 32 * hA:32 * hA + 32].T.reshape(32, 2, 128)
    wp[64:96] = w_proj[:, 32 * hB:32 * hB + 32].T.reshape(32, 2, 128)
    wp = wp.astype(np.float16)

    return {
        "x": np.ascontiguousarray(xb.reshape(2, 128, N)).astype(np.float16),
        "wqk": wqk, "wv": wv, "dg": dg, "bl": bl, "wp": wp,
    }


_LAST_RES = None


def kernel(x, w_qkv, w_lepe, b_lepe, w_proj, b_proj, **_ignored):
    global _LAST_RES
    nc = _get_nc()
    in_maps = [_prep_core(c, x, w_qkv, w_lepe, b_lepe, w_proj)
               for c in range(NCORES)]
    res = run_bass_kernel_spmd(nc, in_maps, core_ids=list(range(NCORES)))
    _LAST_RES = res
    out = np.zeros((B, C, N), np.float32)
    for c in range(NCORES):
        out[c // 4] += res.results[c]["out"].reshape(C, N)
    out += np.asarray(b_proj, np.float32)[None, :, None]
    return out.reshape(B, C, H, W).astype(np.float32)
